# revision 25
# baseline (speedup 1.0000x reference)
"""Masked dot-product attention (d2l DotProductAttention, inference) on 8
Trainium2 NeuronCores via Bass/Tile.

Math: out[b] = softmax(mask(Q[b] @ K[b].T / sqrt(64), valid_lens[b])) @ V[b]
for b in 0..31, seq 2048, head dim 64.

Strategy (v2)
-------------
* Work is decomposed into PIECES: (head, subset of its 128-row k-tiles).
  Only k-tiles below a head's valid_len exist. Pieces are packed into
  NSLOTS uniform segments per core x 8 cores so the single SPMD program's
  baked per-segment tile counts stay load balanced. A head split across
  pieces/cores is recombined on the host by summing the partial
  (unnormalized) numerators and denominators.
* QK^T runs ROW-PAIRED on the PE array (contraction is only d=64, so the
  array's 128 contraction rows hold TWO independent matmuls via
  tile_position=(0,0)/(64,0)): K^T tile weights are duplicated into both
  partition halves; Q^T is reshaped to [128, 1024] with q-columns
  0..1023 in partitions 0..63 and q-columns 1024..2047 in partitions
  64..127. Each 512-column stream serves TWO q-chunks at once => half
  the QK PE time of the naive layout.
* exp (no max-subtraction needed; the reference's -1e6 mask fill makes
  masked exp exactly 0 via the vm mask) is SPLIT across two engines:
  ScalarE native Exp activation, and VectorE via a Schraudolph bit-trick
  (bits_i16 = round(s * 128*log2(e)/8 + B) viewed as bf16 ~= exp(s/8),
  ~2% rms multiplicative error that largely cancels in the softmax
  ratio). The fraction on DVE is a tunable accuracy/perf knob. The DVE
  also does the output drains, so ScalarE runs nearly pure exp.
* Masking + softmax denominator fold into the V operand: vm = [V * m, m]
  (m = 0/1 mask). AV: lhsT = vm tile [128, 65], rhs = P^T chunk
  [128, 512] accumulates O'[d, q] over the segment's k-tiles in a
  [65, 2048] PSUM accumulator; row 64 is the denominator. AV for tile t
  issues after exp of tile t+1 (one-tile software pipeline).
* PSUM budget: score tiles [128,1024] fp32 (2 banks, each half written
  by one of the paired matmuls) x 2 bufs + oacc [65,2048] fp32 (4
  banks) = 8 banks exactly.
* Host post-pass: per head, sum piece partials, out = (num/den).T.
"""

import math
import sys

import numpy as np

for _p in (
    "/root/.axon_site",
    "/root/.axon_site/_ro/trn_rl_repo",
    "/root/.axon_site/_ro/pypackages",
):
    if _p not in sys.path:
        sys.path.append(_p)

import ml_dtypes


def _ensure_axon_hooks_shim():
    """This image's antenv package lacks axon_hooks; bass_utils imports it
    when tracing is requested (e.g. via BASS_TRACE in the environment).
    Provide a null shim so tracing degrades gracefully instead of crashing.
    A harness can set a real hook via set_axon_ntff_profile_hook."""
    import types

    if "antenv.axon_hooks" in sys.modules:
        return
    try:
        import antenv.axon_hooks  # noqa: F401

        return
    except ImportError:
        pass
    import antenv  # noqa: F401

    mod = types.ModuleType("antenv.axon_hooks")
    mod._hook = None
    mod.set_axon_ntff_profile_hook = lambda h: setattr(mod, "_hook", h)
    mod.get_axon_ntff_profile_hook = lambda: mod._hook
    sys.modules["antenv.axon_hooks"] = mod


_ensure_axon_hooks_shim()

import concourse.bacc as bacc
import concourse.mybir as mybir
import concourse.tile as tile
from concourse.bass_utils import run_bass_kernel_spmd

N, S, D = 32, 2048, 64
N_CORES = 8
KTILE = 128           # k rows per tile (PE contraction tile)
VW = D + 1            # V columns + denominator ones-column
HS = S // 2           # 1024: q columns per partition-half

BF16 = mybir.dt.bfloat16
F32 = mybir.dt.float32
I16 = mybir.dt.int16

# Schraudolph bf16-bits exp: bits_i16(exp(s/8)) ~= s * A + B
SCHRAUD_A = 128.0 * math.log2(math.e) / 8.0          # 23.083120654...
SCHRAUD_B = 16256.0 - 5.513                          # 127<<7 minus minimax centering

# Fraction of exp units (one per (segment, half, k-tile)) computed on the
# DVE via the Schraudolph trick; the rest use ScalarE's exact Exp.
EXP_DVE_FRAC = 0.46

TRACE = False          # test.py flips this to profile
LAST_RESULTS = None    # BassKernelResults of the last run

_program_cache: dict = {}


def _dve_pattern(nunits: int, frac: float) -> list[bool]:
    """Evenly spread `frac` of units onto the DVE."""
    out, acc = [], 0.0
    for _ in range(nunits):
        acc += frac
        if acc >= 1.0 - 1e-9:
            acc -= 1.0
            out.append(True)
        else:
            out.append(False)
    return out


def _build_program(T, dve_frac=EXP_DVE_FRAC):
    """One SPMD program; T = per-segment k-tile counts (desc, same on all
    cores)."""
    nslots = len(T)
    maxpt = max(T)
    # One combined per-segment input [128, W]: cols [0, HS) = Q^T
    # (q-halves stacked in partitions), [HS, HS+maxpt*KTILE) = K^T tiles
    # (partition-duplicated), [KOFF_V, KOFF_V+maxpt*VW) = vm. One DMA per
    # segment => few, large descriptors.
    KOFF_V = HS + maxpt * KTILE
    W = KOFF_V + maxpt * VW
    nc = bacc.Bacc("TRN2", target_bir_lowering=False, debug=False)
    qkv = nc.dram_tensor("qkv", [nslots, 2 * D, W], BF16, kind="ExternalInput")
    out = nc.dram_tensor("out", [nslots, VW, S], F32, kind="ExternalOutput")

    exp = mybir.ActivationFunctionType.Exp
    scale = float(1.0 / math.sqrt(D))

    # exp engine assignment, one unit per (segment, half, k-tile). The
    # first few units are forced onto the DVE so ScalarE's one-time
    # activation-table load (~2.7us) stays off the critical path.
    dve_units = _dve_pattern(2 * sum(T), dve_frac)
    dve_units[:3] = [True, True, True]
    unit = 0

    # Flat unit list: one unit per (segment, half, k-tile). Emission is
    # software-pipelined ACROSS half/segment boundaries: unit u emits its
    # QK pair + exp, then the AV of unit u-1 — so the strictly-FIFO PE
    # queue always has the next unit's QK in front of an AV that may
    # still be waiting on its exp.
    units = []
    for j in range(nslots):
        if T[j] > 0:
            for h in range(2):
                for t in range(T[j]):
                    units.append((j, h, t))

    with tile.TileContext(nc) as tc:
        with (
            tc.tile_pool(name="qp", bufs=2) as qp,
            tc.tile_pool(name="pp", bufs=5) as pp,
            tc.tile_pool(name="ob", bufs=2) as ob,
            tc.tile_pool(name="sp", bufs=3, space="PSUM") as sp,
            tc.tile_pool(name="op", bufs=1, space="PSUM") as op,
        ):
            seg = {}    # live segment tiles: j -> xj
            oaccs = {}  # live accumulators: (j, h) -> oacc
            obufs = {}  # live output staging: j -> obuf
            pend = []   # delayed AV emissions: (j, h, oacc, vmj, pt, t, tj)

            def flush_one():
                j, h, oacc, vmj, pt, t, tj = pend.pop(0)
                vcols = slice(KOFF_V + t * VW, KOFF_V + (t + 1) * VW)
                for qq in range(2):
                    nc.tensor.matmul(
                        oacc[:, qq * 512 : (qq + 1) * 512],
                        vmj[:, vcols],
                        pt[:, qq * 512 : (qq + 1) * 512],
                        start=(t == 0),
                        stop=(t == tj - 1),
                    )
                if t == tj - 1:
                    # segment-half complete: drain split across both
                    # engines so the oacc banks free quickly
                    if j not in obufs:
                        obufs[j] = ob.tile([VW, S], F32, tag="ob", name="obuf")
                    obuf = obufs[j]
                    nc.scalar.copy(obuf[:, h * HS : h * HS + 512], oacc[:, 0:512])
                    nc.vector.tensor_copy(
                        obuf[:, h * HS + 512 : (h + 1) * HS], oacc[:, 512:HS]
                    )
                    del oaccs[(j, h)]
                    if h == 1:
                        # one output DMA per segment, on the GpSimd queue
                        # so the Sync queue only carries input prefetches
                        nc.gpsimd.dma_start(out[j], obuf[:, :])
                        del obufs[j]

            for j, h, t in units:
                tj = T[j]
                if j not in seg:
                    xj = qp.tile([2 * D, W], BF16, tag="x", name="xj")
                    if j == 0:
                        # expose the first matmuls' operands first: k-tile
                        # 0 of K^T, then the first q-chunk, then the rest
                        nc.sync.dma_start(
                            xj[:, HS : HS + KTILE], qkv[j, :, HS : HS + KTILE]
                        )
                        nc.sync.dma_start(xj[:, 0:512], qkv[j, :, 0:512])
                        nc.sync.dma_start(xj[:, 512:HS], qkv[j, :, 512:HS])
                        nc.sync.dma_start(
                            xj[:, HS + KTILE : W], qkv[j, :, HS + KTILE : W]
                        )
                    else:
                        nc.sync.dma_start(xj[:, :], qkv[j])
                    seg[j] = xj
                xj = seg[j]
                qtj = xj
                ktj = xj
                vmj = xj
                if (j, h) not in oaccs:
                    oaccs[(j, h)] = op.tile([VW, HS], F32, tag="o", name="oacc")
                oacc = oaccs[(j, h)]

                qcols = slice(512 * h, 512 * h + 512)
                kcols = slice(HS + t * KTILE, HS + (t + 1) * KTILE)
                ps = sp.tile([KTILE, 2 * 512], F32, tag="s")
                pt = pp.tile([KTILE, 2 * 512], BF16, tag="p")
                # row-paired QK: both matmuls run concurrently in the PE
                # array (contraction rows 0-63 / 64-127)
                nc.tensor.matmul(
                    ps[:, 0:512], ktj[0:D, kcols], qtj[0:D, qcols],
                    start=True, stop=True, tile_position=(0, 0),
                )
                nc.tensor.matmul(
                    ps[:, 512:1024], ktj[D : 2 * D, kcols],
                    qtj[D : 2 * D, qcols],
                    start=True, stop=True, tile_position=(64, 0),
                )
                if dve_units[unit]:
                    nc.vector.tensor_scalar(
                        pt[:, :].bitcast(I16), ps[:, :],
                        SCHRAUD_A, SCHRAUD_B,
                        mybir.AluOpType.mult, mybir.AluOpType.add,
                    )
                else:
                    nc.scalar.activation(pt[:, :], ps[:, :], exp, scale=scale)
                unit += 1
                if pend:
                    flush_one()
                pend.append((j, h, oacc, vmj, pt, t, tj))
            while pend:
                flush_one()
    nc.compile()
    return nc


def _emit_av(nc, oacc, vmj, pt, t, tj):
    vcols = slice(t * VW, (t + 1) * VW)
    for qq in range(2):
        nc.tensor.matmul(
            oacc[:, qq * 512 : (qq + 1) * 512],
            vmj[:, vcols],
            pt[:, qq * 512 : (qq + 1) * 512],
            start=(t == 0),
            stop=(t == tj - 1),
        )


def _pack_pieces(tiles_per_head):
    """Split heads into 8*nslots pieces, minimizing C = sum of per-slot
    maxima (the baked per-core tile count). Returns (slot_sizes, pieces)
    where pieces[j][c] = (head, [tile indices]) for slot j, core c."""
    nheads = len(tiles_per_head)
    best = None
    for nslots in range(max(1, nheads // 8), nheads // 8 + 8):
        npieces = 8 * nslots
        n = dict.fromkeys(range(nheads), 1)

        def maxpiece(h):
            return math.ceil(tiles_per_head[h] / n[h])

        for _ in range(npieces - nheads):
            h = max(range(nheads), key=lambda h: (maxpiece(h), tiles_per_head[h]))
            if maxpiece(h) <= 1:
                break
            n[h] += 1
        pieces = []
        for h in range(nheads):
            nh = n[h]
            q, r = divmod(tiles_per_head[h], nh)
            start = 0
            for i in range(nh):
                sz = q + 1 if i < r else q
                if sz > 0:
                    pieces.append((sz, h, list(range(start, start + sz))))
                start += sz
        pieces.sort(key=lambda p: -p[0])
        while len(pieces) < npieces:
            pieces.append((0, -1, []))
        slot_sizes = tuple(pieces[8 * j][0] for j in range(nslots))
        C = sum(slot_sizes)
        # penalty per extra slot for segment-drain + output-DMA overhead
        cost = C + 0.6 * nslots
        if best is None or cost < best[0]:
            best = (cost, slot_sizes, pieces)
    _, slot_sizes, pieces = best
    nslots = len(slot_sizes)
    grid = [[pieces[8 * j + c] for c in range(8)] for j in range(nslots)]
    return slot_sizes, grid


def kernel(queries, keys, values, valid_lens):
    global LAST_RESULTS
    queries = np.asarray(queries, dtype=np.float32)
    keys = np.asarray(keys, dtype=np.float32)
    values = np.asarray(values, dtype=np.float32)
    vl = np.asarray(valid_lens).astype(np.int64)
    assert queries.shape == (N, S, D) and vl.shape == (N,)

    tiles_per_head = [max(1, int(math.ceil(int(v) / KTILE))) for v in vl]
    slot_sizes, grid = _pack_pieces(tiles_per_head)
    nslots = len(slot_sizes)
    maxpt = max(slot_sizes)

    nc = _program_cache.get(slot_sizes)
    if nc is None:
        nc = _build_program(slot_sizes)
        _program_cache[slot_sizes] = nc

    bf = ml_dtypes.bfloat16
    # qt_all[h]: [128, 1024]; partitions 0-63 = Q^T cols 0-1023,
    # partitions 64-127 = Q^T cols 1024-2047
    qt_full = np.ascontiguousarray(queries.transpose(0, 2, 1)).astype(bf)  # [N,64,S]
    qt_all = np.concatenate([qt_full[:, :, 0:HS], qt_full[:, :, HS:S]], axis=1)
    kt_half = np.ascontiguousarray(keys.transpose(0, 2, 1)).astype(bf)    # [N,64,S]
    kt_all = np.concatenate([kt_half, kt_half], axis=1)                   # [N,128,S]
    # vm_all[h]: [KTILE, 16, VW]  (partition-major tiling of [V*m, m])
    vm_all = np.zeros((N, KTILE, S // KTILE, VW), dtype=bf)
    for h in range(N):
        m = (np.arange(S) < vl[h]).astype(np.float32)
        vp_full = np.concatenate([values[h] * m[:, None], m[:, None]], axis=1)
        vm_all[h] = vp_full.reshape(S // KTILE, KTILE, VW).transpose(1, 0, 2).astype(bf)

    KOFF_V = HS + maxpt * KTILE
    W = KOFF_V + maxpt * VW
    in_maps = []
    for c in range(N_CORES):
        x_c = np.zeros((nslots, 2 * D, W), dtype=bf)
        for j in range(nslots):
            sz, h, tidx = grid[j][c]
            if sz == 0:
                continue
            x_c[j, :, 0:HS] = qt_all[h]
            for i, t in enumerate(tidx):
                x_c[j, :, HS + i * KTILE : HS + (i + 1) * KTILE] = kt_all[
                    h, :, t * KTILE : (t + 1) * KTILE
                ]
                x_c[j, :, KOFF_V + i * VW : KOFF_V + (i + 1) * VW] = vm_all[h, :, t, :]
        in_maps.append({"qkv": x_c})

    res = run_bass_kernel_spmd(nc, in_maps, core_ids=list(range(N_CORES)), trace=TRACE)
    LAST_RESULTS = res

    acc = np.zeros((N, VW, S), dtype=np.float64)
    for c in range(N_CORES):
        o = res.results[c]["out"]  # [nslots, 65, S] fp32
        for j in range(nslots):
            sz, h, _ = grid[j][c]
            if sz > 0:
                acc[h] += o[j]
    # device column blocks are [c0, c2, c1, c3]; natural q-chunk c lives
    # at device block [0, 2, 1, 3][c]
    perm = np.concatenate(
        [np.arange(512), np.arange(1024, 1536), np.arange(512, 1024),
         np.arange(1536, 2048)]
    )
    acc = acc[:, :, perm]
    out_full = (acc[:, :D, :] / acc[:, D : D + 1, :]).transpose(0, 2, 1)
    return np.ascontiguousarray(out_full.astype(np.float32))


# revision 28
# speedup vs baseline: 1.0478x; 1.0478x over previous
"""Masked dot-product attention (d2l DotProductAttention, inference) on 8
Trainium2 NeuronCores via Bass/Tile.

Math: out[b] = softmax(mask(Q[b] @ K[b].T / sqrt(64), valid_lens[b])) @ V[b]
for b in 0..31, seq 2048, head dim 64.

Strategy (v2)
-------------
* Work is decomposed into PIECES: (head, subset of its 128-row k-tiles).
  Only k-tiles below a head's valid_len exist. Pieces are packed into
  NSLOTS uniform segments per core x 8 cores so the single SPMD program's
  baked per-segment tile counts stay load balanced. A head split across
  pieces/cores is recombined on the host by summing the partial
  (unnormalized) numerators and denominators.
* QK^T runs ROW-PAIRED on the PE array (contraction is only d=64, so the
  array's 128 contraction rows hold TWO independent matmuls via
  tile_position=(0,0)/(64,0)): K^T tile weights are duplicated into both
  partition halves; Q^T is reshaped to [128, 1024] with q-columns
  0..1023 in partitions 0..63 and q-columns 1024..2047 in partitions
  64..127. Each 512-column stream serves TWO q-chunks at once => half
  the QK PE time of the naive layout.
* exp (no max-subtraction needed; the reference's -1e6 mask fill makes
  masked exp exactly 0 via the vm mask) is SPLIT across two engines:
  ScalarE native Exp activation, and VectorE via a Schraudolph bit-trick
  (bits_i16 = round(s * 128*log2(e)/8 + B) viewed as bf16 ~= exp(s/8),
  ~2% rms multiplicative error that largely cancels in the softmax
  ratio). The fraction on DVE is a tunable accuracy/perf knob. The DVE
  also does the output drains, so ScalarE runs nearly pure exp.
* Masking + softmax denominator fold into the V operand: vm = [V * m, m]
  (m = 0/1 mask). AV: lhsT = vm tile [128, 65], rhs = P^T chunk
  [128, 512] accumulates O'[d, q] over the segment's k-tiles in a
  [65, 2048] PSUM accumulator; row 64 is the denominator. AV for tile t
  issues after exp of tile t+1 (one-tile software pipeline).
* PSUM budget: score tiles [128,1024] fp32 (2 banks, each half written
  by one of the paired matmuls) x 2 bufs + oacc [65,2048] fp32 (4
  banks) = 8 banks exactly.
* Host post-pass: per head, sum piece partials, out = (num/den).T.
"""

import math
import sys

import numpy as np

for _p in (
    "/root/.axon_site",
    "/root/.axon_site/_ro/trn_rl_repo",
    "/root/.axon_site/_ro/pypackages",
):
    if _p not in sys.path:
        sys.path.append(_p)

import ml_dtypes


def _ensure_axon_hooks_shim():
    """This image's antenv package lacks axon_hooks; bass_utils imports it
    when tracing is requested (e.g. via BASS_TRACE in the environment).
    Provide a null shim so tracing degrades gracefully instead of crashing.
    A harness can set a real hook via set_axon_ntff_profile_hook."""
    import types

    if "antenv.axon_hooks" in sys.modules:
        return
    try:
        import antenv.axon_hooks  # noqa: F401

        return
    except ImportError:
        pass
    import antenv  # noqa: F401

    mod = types.ModuleType("antenv.axon_hooks")
    mod._hook = None
    mod.set_axon_ntff_profile_hook = lambda h: setattr(mod, "_hook", h)
    mod.get_axon_ntff_profile_hook = lambda: mod._hook
    sys.modules["antenv.axon_hooks"] = mod


_ensure_axon_hooks_shim()

import concourse.bacc as bacc
import concourse.mybir as mybir
import concourse.tile as tile
from concourse.bass_utils import run_bass_kernel_spmd

N, S, D = 32, 2048, 64
N_CORES = 8
KTILE = 128           # k rows per tile (PE contraction tile)
VW = D + 1            # V columns + denominator ones-column
HS = S // 2           # 1024: q columns per partition-half

BF16 = mybir.dt.bfloat16
F32 = mybir.dt.float32
I16 = mybir.dt.int16

# Schraudolph bf16-bits exp: bits_i16(exp(s/8)) ~= s * A + B
SCHRAUD_A = 128.0 * math.log2(math.e) / 8.0          # 23.083120654...
SCHRAUD_B = 16256.0 - 5.513                          # 127<<7 minus minimax centering

# Fraction of exp units (one per (segment, half, k-tile)) computed on the
# DVE via the Schraudolph trick; the rest use ScalarE's exact Exp.
EXP_DVE_FRAC = 0.46

TRACE = False          # test.py flips this to profile
LAST_RESULTS = None    # BassKernelResults of the last run

_program_cache: dict = {}


def _dve_pattern(nunits: int, frac: float) -> list[bool]:
    """Evenly spread `frac` of units onto the DVE."""
    out, acc = [], 0.0
    for _ in range(nunits):
        acc += frac
        if acc >= 1.0 - 1e-9:
            acc -= 1.0
            out.append(True)
        else:
            out.append(False)
    return out


def _build_program(T, dve_frac=EXP_DVE_FRAC):
    """One SPMD program; T = per-segment k-tile counts (desc, same on all
    cores)."""
    nslots = len(T)
    maxpt = max(T)
    # One combined per-segment input [128, W]: cols [0, HS) = Q^T
    # (q-halves stacked in partitions), [HS, HS+maxpt*KTILE) = K^T tiles
    # (partition-duplicated), [KOFF_V, KOFF_V+maxpt*VW) = vm. One DMA per
    # segment => few, large descriptors.
    KOFF_V = HS + maxpt * KTILE
    W = KOFF_V + maxpt * VW
    nc = bacc.Bacc("TRN2", target_bir_lowering=False, debug=False)
    qkv = nc.dram_tensor("qkv", [nslots, 2 * D, W], BF16, kind="ExternalInput")
    out = nc.dram_tensor("out", [nslots, VW, S], F32, kind="ExternalOutput")

    exp = mybir.ActivationFunctionType.Exp
    scale = float(1.0 / math.sqrt(D))

    # exp engine assignment, one unit per (segment, half, k-tile, chunk).
    # The first few units are forced onto the DVE so ScalarE's one-time
    # activation-table load (~2.7us) stays off the critical path.
    dve_units = _dve_pattern(4 * sum(T), dve_frac)
    dve_units[:4] = [True, True, True, True]
    unit = 0

    # Flat unit list: one unit per (segment, half, k-tile). Emission is
    # software-pipelined ACROSS half/segment boundaries: unit u emits its
    # QK pair + exp, then the AV of unit u-1 — so the strictly-FIFO PE
    # queue always has the next unit's QK in front of an AV that may
    # still be waiting on its exp.
    units = []
    for j in range(nslots):
        if T[j] > 0:
            for h in range(2):
                for t in range(T[j]):
                    units.append((j, h, t))

    with tile.TileContext(nc) as tc:
        with (
            tc.tile_pool(name="qp", bufs=2) as qp,
            tc.tile_pool(name="pp", bufs=5) as pp,
            tc.tile_pool(name="ob", bufs=2) as ob,
            tc.tile_pool(name="sp", bufs=3, space="PSUM") as sp,
            tc.tile_pool(name="op", bufs=1, space="PSUM") as op,
        ):
            seg = {}    # live segment tiles: j -> xj
            oaccs = {}  # live accumulators: (j, h) -> oacc
            obufs = {}  # live output staging: j -> obuf
            pend = []   # delayed AV emissions: (j, h, oacc, vmj, pt, t, tj)

            def flush_one():
                j, h, oacc, vmj, pt_a, pt_b, t, tj = pend.pop(0)
                vcols = slice(KOFF_V + t * VW, KOFF_V + (t + 1) * VW)
                for qq, pt in ((0, pt_a), (1, pt_b)):
                    nc.tensor.matmul(
                        oacc[:, qq * 512 : (qq + 1) * 512],
                        vmj[:, vcols],
                        pt[:, :],
                        start=(t == 0),
                        stop=(t == tj - 1),
                    )
                if t == tj - 1:
                    # segment-half complete: drain split across both
                    # engines so the oacc banks free quickly
                    obuf = ob.tile([VW, HS], F32, tag="ob", name="obuf")
                    nc.scalar.copy(obuf[:, 0:512], oacc[:, 0:512])
                    nc.vector.tensor_copy(obuf[:, 512:HS], oacc[:, 512:HS])
                    del oaccs[(j, h)]
                    # output DMA per half, on the GpSimd queue so the
                    # Sync queue only carries input prefetches
                    nc.gpsimd.dma_start(
                        out[j, :, h * HS : (h + 1) * HS], obuf[:, :]
                    )

            for j, h, t in units:
                tj = T[j]
                if j not in seg:
                    xj = qp.tile([2 * D, W], BF16, tag="x", name="xj")
                    if j == 0:
                        # expose the first matmuls' operands first: k-tile
                        # 0 of K^T, then the first q-chunk, then the rest
                        nc.sync.dma_start(
                            xj[:, HS : HS + KTILE], qkv[j, :, HS : HS + KTILE]
                        )
                        nc.sync.dma_start(xj[:, 0:512], qkv[j, :, 0:512])
                        nc.sync.dma_start(xj[:, 512:HS], qkv[j, :, 512:HS])
                        nc.sync.dma_start(
                            xj[:, HS + KTILE : W], qkv[j, :, HS + KTILE : W]
                        )
                    else:
                        nc.sync.dma_start(xj[:, :], qkv[j])
                    seg[j] = xj
                xj = seg[j]
                qtj = xj
                ktj = xj
                vmj = xj
                if (j, h) not in oaccs:
                    oaccs[(j, h)] = op.tile([VW, HS], F32, tag="o", name="oacc")
                oacc = oaccs[(j, h)]

                qcols = slice(512 * h, 512 * h + 512)
                kcols = slice(HS + t * KTILE, HS + (t + 1) * KTILE)
                ps_a = sp.tile([KTILE, 512], F32, tag="sa", name="ps_a")
                ps_b = sp.tile([KTILE, 512], F32, tag="sb", name="ps_b")
                pt_a = pp.tile([KTILE, 512], BF16, tag="pa", name="pt_a")
                pt_b = pp.tile([KTILE, 512], BF16, tag="pb", name="pt_b")
                # row-paired QK: both matmuls run concurrently in the PE
                # array (contraction rows 0-63 / 64-127)
                nc.tensor.matmul(
                    ps_a[:, :], ktj[0:D, kcols], qtj[0:D, qcols],
                    start=True, stop=True, tile_position=(0, 0),
                )
                nc.tensor.matmul(
                    ps_b[:, :], ktj[D : 2 * D, kcols],
                    qtj[D : 2 * D, qcols],
                    start=True, stop=True, tile_position=(64, 0),
                )
                # per-chunk exp: two independent FD=512 calls with their
                # own engine assignment — shorter latency chains, and the
                # two PSUM banks recycle independently
                for ps, pt in ((ps_a, pt_a), (ps_b, pt_b)):
                    if dve_units[unit % len(dve_units)]:
                        nc.vector.tensor_scalar(
                            pt[:, :].bitcast(I16), ps[:, :],
                            SCHRAUD_A, SCHRAUD_B,
                            mybir.AluOpType.mult, mybir.AluOpType.add,
                        )
                    else:
                        nc.scalar.activation(pt[:, :], ps[:, :], exp, scale=scale)
                    unit += 1
                if pend:
                    flush_one()
                pend.append((j, h, oacc, vmj, pt_a, pt_b, t, tj))
            while pend:
                flush_one()
    nc.compile()
    return nc


def _emit_av(nc, oacc, vmj, pt, t, tj):
    vcols = slice(t * VW, (t + 1) * VW)
    for qq in range(2):
        nc.tensor.matmul(
            oacc[:, qq * 512 : (qq + 1) * 512],
            vmj[:, vcols],
            pt[:, qq * 512 : (qq + 1) * 512],
            start=(t == 0),
            stop=(t == tj - 1),
        )


def _pack_pieces(tiles_per_head):
    """Split heads into 8*nslots pieces, minimizing C = sum of per-slot
    maxima (the baked per-core tile count). Returns (slot_sizes, pieces)
    where pieces[j][c] = (head, [tile indices]) for slot j, core c."""
    nheads = len(tiles_per_head)
    best = None
    for nslots in range(max(1, nheads // 8), nheads // 8 + 8):
        npieces = 8 * nslots
        n = dict.fromkeys(range(nheads), 1)

        def maxpiece(h):
            return math.ceil(tiles_per_head[h] / n[h])

        for _ in range(npieces - nheads):
            h = max(range(nheads), key=lambda h: (maxpiece(h), tiles_per_head[h]))
            if maxpiece(h) <= 1:
                break
            n[h] += 1
        pieces = []
        for h in range(nheads):
            nh = n[h]
            q, r = divmod(tiles_per_head[h], nh)
            start = 0
            for i in range(nh):
                sz = q + 1 if i < r else q
                if sz > 0:
                    pieces.append((sz, h, list(range(start, start + sz))))
                start += sz
        pieces.sort(key=lambda p: -p[0])
        while len(pieces) < npieces:
            pieces.append((0, -1, []))
        slot_sizes = tuple(pieces[8 * j][0] for j in range(nslots))
        C = sum(slot_sizes)
        # penalty per extra slot for segment-drain + output-DMA overhead
        cost = C + 0.6 * nslots
        if best is None or cost < best[0]:
            best = (cost, slot_sizes, pieces)
    _, slot_sizes, pieces = best
    nslots = len(slot_sizes)
    grid = [[pieces[8 * j + c] for c in range(8)] for j in range(nslots)]
    return slot_sizes, grid


def kernel(queries, keys, values, valid_lens):
    global LAST_RESULTS
    queries = np.asarray(queries, dtype=np.float32)
    keys = np.asarray(keys, dtype=np.float32)
    values = np.asarray(values, dtype=np.float32)
    vl = np.asarray(valid_lens).astype(np.int64)
    assert queries.shape == (N, S, D) and vl.shape == (N,)

    tiles_per_head = [max(1, int(math.ceil(int(v) / KTILE))) for v in vl]
    slot_sizes, grid = _pack_pieces(tiles_per_head)
    nslots = len(slot_sizes)
    maxpt = max(slot_sizes)

    nc = _program_cache.get(slot_sizes)
    if nc is None:
        nc = _build_program(slot_sizes)
        _program_cache[slot_sizes] = nc

    bf = ml_dtypes.bfloat16
    # qt_all[h]: [128, 1024]; partitions 0-63 = Q^T cols 0-1023,
    # partitions 64-127 = Q^T cols 1024-2047
    qt_full = np.ascontiguousarray(queries.transpose(0, 2, 1)).astype(bf)  # [N,64,S]
    qt_all = np.concatenate([qt_full[:, :, 0:HS], qt_full[:, :, HS:S]], axis=1)
    kt_half = np.ascontiguousarray(keys.transpose(0, 2, 1)).astype(bf)    # [N,64,S]
    kt_all = np.concatenate([kt_half, kt_half], axis=1)                   # [N,128,S]
    # vm_all[h]: [KTILE, 16, VW]  (partition-major tiling of [V*m, m])
    vm_all = np.zeros((N, KTILE, S // KTILE, VW), dtype=bf)
    for h in range(N):
        m = (np.arange(S) < vl[h]).astype(np.float32)
        vp_full = np.concatenate([values[h] * m[:, None], m[:, None]], axis=1)
        vm_all[h] = vp_full.reshape(S // KTILE, KTILE, VW).transpose(1, 0, 2).astype(bf)

    KOFF_V = HS + maxpt * KTILE
    W = KOFF_V + maxpt * VW
    in_maps = []
    for c in range(N_CORES):
        x_c = np.zeros((nslots, 2 * D, W), dtype=bf)
        for j in range(nslots):
            sz, h, tidx = grid[j][c]
            if sz == 0:
                continue
            x_c[j, :, 0:HS] = qt_all[h]
            for i, t in enumerate(tidx):
                x_c[j, :, HS + i * KTILE : HS + (i + 1) * KTILE] = kt_all[
                    h, :, t * KTILE : (t + 1) * KTILE
                ]
                x_c[j, :, KOFF_V + i * VW : KOFF_V + (i + 1) * VW] = vm_all[h, :, t, :]
        in_maps.append({"qkv": x_c})

    res = run_bass_kernel_spmd(nc, in_maps, core_ids=list(range(N_CORES)), trace=TRACE)
    LAST_RESULTS = res

    acc = np.zeros((N, VW, S), dtype=np.float64)
    for c in range(N_CORES):
        o = res.results[c]["out"]  # [nslots, 65, S] fp32
        for j in range(nslots):
            sz, h, _ = grid[j][c]
            if sz > 0:
                acc[h] += o[j]
    # device column blocks are [c0, c2, c1, c3]; natural q-chunk c lives
    # at device block [0, 2, 1, 3][c]
    perm = np.concatenate(
        [np.arange(512), np.arange(1024, 1536), np.arange(512, 1024),
         np.arange(1536, 2048)]
    )
    acc = acc[:, :, perm]
    out_full = (acc[:, :D, :] / acc[:, D : D + 1, :]).transpose(0, 2, 1)
    return np.ascontiguousarray(out_full.astype(np.float32))
